# revision 1
# baseline (speedup 1.0000x reference)
"""Distributed kernel for nn_CSNMModule_38663295598699 (sparse_attention).

Sharding: pure data parallel over B across the 8 trn2 NeuronCores — one
sample per core, all params (~1.8M floats) replicated. The pairwise
pool+MLP+softmax pipeline is independent per sample; only the trivial
[B, D] gather at the end crosses cores (done host-side).

Self-contained: shapes hardcoded (B=8, N=4096, D=512, grids prod 4096).
"""

import functools

import jax
import jax.numpy as jnp
import numpy as np

GRIDS = ((16, 16, 16), (32, 16, 8), (8, 32, 16))  # prod == 4096 for each scale
PAIRS = ((0, 1), (0, 2), (1, 2))
KVOL = 27.0  # 3x3x3 kernel, count_include_pad=True -> always divide by 27

B, N, D = 8, 4096, 512
N_CORES = 8


def _avg_pool3(x):
    # x: [C, d, h, w]; avg_pool3d kernel 3, stride 1, pad 1 (shape preserving),
    # separable: 3-tap sum along each grid dim, then divide by 27.
    for axis in (1, 2, 3):
        pad_lo = [(0, 0)] * 4
        pad_lo[axis] = (1, 0)
        pad_hi = [(0, 0)] * 4
        pad_hi[axis] = (0, 1)
        lo = jnp.pad(x, pad_lo)[
            tuple(slice(None) if a != axis else slice(0, x.shape[axis]) for a in range(4))
        ]
        hi = jnp.pad(x, pad_hi)[
            tuple(slice(None) if a != axis else slice(1, x.shape[axis] + 1) for a in range(4))
        ]
        x = lo + x + hi
    return x / KVOL


def _window(feat, grid):
    # feat: [N, D] -> raw reshape into [C, d, h, w] (layout-mixing, faithful
    # to torch .reshape), pool, then back to [N, D].
    x = feat.reshape(-1, *grid)
    p = _avg_pool3(x)
    return p.reshape(p.shape[0], -1).T  # [N, D]


def _per_sample(e0, e1, e2, W1, b1, W2, b2, gamma, beta, Wf, bf):
    # e0/e1/e2: [N, D] single-sample shards.
    embeds = (e0, e1, e2)
    windows = [_window(embeds[s], GRIDS[s]) for s in range(3)]  # each [N, D]
    matched = []
    for k, (i, j) in enumerate(PAIRS):
        def mlp(c):
            h = jax.nn.relu(c @ W1[k] + b1[k])
            return h @ W2[k] + b2[k]  # [N, 1]

        cij = jnp.concatenate([windows[i], windows[j]], axis=-1)  # [N, 2D]
        cji = jnp.concatenate([windows[j], windows[i]], axis=-1)
        aij = jax.nn.softmax(mlp(cij), axis=0)
        aji = jax.nn.softmax(mlp(cji), axis=0)
        matched.append(jnp.sum(aij * embeds[j], axis=0, keepdims=True))  # [1, D]
        matched.append(jnp.sum(aji * embeds[i], axis=0, keepdims=True))
    fused = jnp.concatenate(list(embeds) + matched, axis=0).mean(axis=0)  # [D]
    mu = fused.mean(keepdims=True)
    var = jnp.mean((fused - mu) ** 2, keepdims=True)
    ln = (fused - mu) / jnp.sqrt(var + 1e-5) * gamma + beta
    return ln @ Wf + bf  # [D]


@functools.cache
def _compiled():
    devices = jax.devices()[:N_CORES]
    return jax.pmap(
        _per_sample,
        in_axes=(0, 0, 0, None, None, None, None, None, None, None, None),
        devices=devices,
    )


def kernel(e0, e1, e2, W1, b1, W2, b2, gamma, beta, Wf, bf):
    # Shard: sample b -> core b (pure data parallel over B).
    fn = _compiled()
    out = fn(
        jnp.asarray(e0), jnp.asarray(e1), jnp.asarray(e2),
        jnp.asarray(W1), jnp.asarray(b1), jnp.asarray(W2), jnp.asarray(b2),
        jnp.asarray(gamma), jnp.asarray(beta), jnp.asarray(Wf), jnp.asarray(bf),
    )
    return np.asarray(out).astype(np.float32)  # [B, D]

